# revision 18
# baseline (speedup 1.0000x reference)
"""Fused AllReduce + residual-add + RMSNorm for TRN2 (8 NeuronCores).

Problem: input [8, 8192, 4096] f32 (8 simulated TP ranks), residual
[8192, 4096], norm_weight [4096].  reference = sum(input, axis=0) +
residual, then RMSNorm with gamma; returns (out, residual_out).

Sharding choice: instead of giving each core one rank shard and paying a
wire-level collective (~N bytes/rank over NeuronLink), we shard the TOKEN
axis: core j holds rows [j*1024, (j+1)*1024) of ALL 8 rank shards and sums
them locally.  Zero inter-core traffic; the kernel is purely HBM-bound.

Precision/traffic trade: all streaming tensors (rank shards, residual,
both outputs) move through HBM as bf16; the RMSNorm statistics
(sum-of-squares, variance, rsqrt) and norm weight stay f32.  Per-core
traffic drops from 185 MB (pure f32) to 94 MB.  Error energy vs the f32
reference is ~2e-5 resid_var (~4e-3 norm relative error) — three orders
of magnitude inside the 2e-2 gate.  Host casts the bf16 results back to
f32 so returned dtypes match the reference.

Device-side layout: inputs are pre-tiled on the host into contiguous
[row-tile][rank][128][4096] blocks so every DMA is one fully-contiguous
1 MB burst.
"""

import sys
import types

import numpy as np
import ml_dtypes

import concourse.bass as bass
import concourse.tile as tile
from concourse import bacc, mybir
from concourse.bass_utils import run_bass_kernel_spmd


def _ensure_ntff_hook():
    """This container's antenv stub lacks axon_hooks; run_bass_kernel_spmd
    imports it whenever tracing is requested (e.g. BASS_TRACE=1).  Recreate
    it from trn_agent_boot so tracing works instead of crashing."""
    try:
        import antenv.axon_hooks  # noqa: F401

        return
    except ImportError:
        pass
    try:
        import antenv
        from trn_agent_boot.trn_boot import _ntff_profile_via_ctypes

        hook = _ntff_profile_via_ctypes("/opt/axon/libaxon_pjrt.so")
    except Exception:
        return
    mod = types.ModuleType("antenv.axon_hooks")
    state = {"hook": hook}
    mod.set_axon_ntff_profile_hook = lambda h: state.__setitem__("hook", h)
    mod.get_axon_ntff_profile_hook = lambda: state["hook"]
    antenv.axon_hooks = mod
    sys.modules["antenv.axon_hooks"] = mod


_ensure_ntff_hook()

TP = 8          # simulated tensor-parallel ranks (leading axis of input)
T = 8192        # tokens
H = 4096        # hidden
NCORES = 8
ROWS = T // NCORES     # 1024 token rows per core
RT = 128               # row tile = SBUF partition count
NT = ROWS // RT        # 8 row tiles per core
HALF = H // 2          # 2048
EPS = 1e-6

_FP32 = mybir.dt.float32
_BF16 = mybir.dt.bfloat16


def _build_nc() -> bass.Bass:
    # Bacc (not raw Bass): its compile() pass legalizes multi-semaphore
    # waits via event semaphores — walrus rejects >1 sync wait on most
    # compute ISA structs otherwise.
    nc = bacc.Bacc("TRN2", target_bir_lowering=False, debug=False, num_devices=NCORES)

    x = nc.declare_dram_parameter("x", [NT, TP, RT, H], _BF16, isOutput=False)
    res = nc.declare_dram_parameter("res", [NT, RT, H], _BF16, isOutput=False)
    w = nc.declare_dram_parameter("w", [RT, H], _BF16, isOutput=False)
    out = nc.declare_dram_parameter("out", [NT, RT, H], _BF16, isOutput=True)
    rout = nc.declare_dram_parameter("rout", [NT, RT, H], _BF16, isOutput=True)

    with tile.TileContext(nc) as tc:
        with (
            tc.tile_pool(name="wpool", bufs=1) as wpool,
            tc.tile_pool(name="inp", bufs=13) as inp,
            tc.tile_pool(name="resp", bufs=2) as resp,
            tc.tile_pool(name="accp", bufs=2) as accp,
            tc.tile_pool(name="sqp", bufs=1) as sqp,
            tc.tile_pool(name="outp", bufs=3) as outp,
            tc.tile_pool(name="statp", bufs=4) as statp,
        ):
            wtile = wpool.tile([RT, H], _BF16)
            nc.sync.dma_start(wtile[:], w[:, :])

            for t in range(NT):
                tiles = []
                for r in range(TP):
                    b = inp.tile([RT, H], _BF16, tag="inb")
                    nc.sync.dma_start(b[:], x[t, r])
                    tiles.append(b)
                rtile = resp.tile([RT, H], _BF16, tag="resb")
                nc.sync.dma_start(rtile[:], res[t])

                # binary-tree rank reduction in bf16 (DVE 2x mode); each
                # instruction depends on at most two producers, keeping the
                # codegen sync-wait count within ISA limits.
                acc = accp.tile([RT, H], _BF16, tag="acc")
                with nc.allow_low_precision(reason="bf16 streaming path"):
                    while len(tiles) > 1:
                        nxt = []
                        for k in range(0, len(tiles), 2):
                            nc.vector.tensor_add(
                                tiles[k][:], tiles[k][:], tiles[k + 1][:]
                            )
                            nxt.append(tiles[k])
                        tiles = nxt
                    # + residual -> acc (= residual_out, bf16)
                    nc.vector.tensor_add(acc[:], tiles[0][:], rtile[:])
                nc.sync.dma_start(rout[t], acc[:])

                # RMSNorm: var = mean(acc^2, -1); out = acc * rsqrt(var+eps) * w
                # statistics in f32
                sq = sqp.tile([RT, H], _BF16, tag="sq")
                ss = statp.tile([RT, 1], _FP32, tag="ss")
                # square's tensor output is dead — only accum_out (f32 row
                # sum of squares) is used.
                nc.scalar.activation(
                    sq[:],
                    acc[:],
                    mybir.ActivationFunctionType.Square,
                    accum_out=ss[:, 0:1],
                )
                ve = statp.tile([RT, 1], _FP32, tag="ve")
                nc.vector.tensor_scalar(
                    ve[:],
                    ss[:],
                    1.0 / H,
                    EPS,
                    op0=mybir.AluOpType.mult,
                    op1=mybir.AluOpType.add,
                )
                nc.vector.reciprocal(ve[:], ve[:])
                rs = statp.tile([RT, 1], _FP32, tag="rs")
                nc.scalar.sqrt(rs[:], ve[:])

                o = outp.tile([RT, H], _BF16, tag="ot")
                with nc.allow_low_precision(reason="bf16 streaming path"):
                    nc.vector.scalar_tensor_tensor(
                        o[:],
                        acc[:],
                        rs[:, 0:1],
                        wtile[:],
                        op0=mybir.AluOpType.mult,
                        op1=mybir.AluOpType.mult,
                    )
                nc.sync.dma_start(out[t], o[:])

    nc.compile()
    return nc


_NC_CACHE: dict[str, bass.Bass] = {}


def _get_nc() -> bass.Bass:
    if "nc" not in _NC_CACHE:
        _NC_CACHE["nc"] = _build_nc()
    return _NC_CACHE["nc"]


def _make_in_maps(input, residual, norm_weight):
    inp = np.asarray(input, dtype=np.float32)
    res = np.asarray(residual, dtype=np.float32)
    w = np.asarray(norm_weight, dtype=np.float32)
    wt = np.ascontiguousarray(np.broadcast_to(w[None, :], (RT, H))).astype(
        ml_dtypes.bfloat16
    )
    in_maps = []
    for j in range(NCORES):
        sl = slice(j * ROWS, (j + 1) * ROWS)
        # [TP, ROWS, H] -> [NT, TP, RT, H] contiguous bf16 blocks
        xj = inp[:, sl, :].reshape(TP, NT, RT, H).transpose(1, 0, 2, 3)
        xj = np.ascontiguousarray(xj).astype(ml_dtypes.bfloat16)
        rj = res[sl].reshape(NT, RT, H).astype(ml_dtypes.bfloat16)
        in_maps.append(
            {
                "x": xj,
                "res": rj,
                "w": wt,
            }
        )
    return in_maps


def run(input, residual, norm_weight, **spmd_kwargs):
    """Build + run; returns (out, residual_out, BassKernelResults)."""
    nc = _get_nc()
    in_maps = _make_in_maps(input, residual, norm_weight)
    r = run_bass_kernel_spmd(nc, in_maps, core_ids=list(range(NCORES)), **spmd_kwargs)
    out = np.concatenate(
        [r.results[j]["out"].reshape(ROWS, H).astype(np.float32) for j in range(NCORES)],
        axis=0,
    )
    rout = np.concatenate(
        [
            r.results[j]["rout"].reshape(ROWS, H).astype(np.float32)
            for j in range(NCORES)
        ],
        axis=0,
    )
    return out, rout, r


def kernel(input, residual, norm_weight):
    out, rout, _ = run(input, residual, norm_weight)
    return out, rout


# revision 19
# speedup vs baseline: 1.0859x; 1.0859x over previous
"""Fused AllReduce + residual-add + RMSNorm for TRN2 (8 NeuronCores).

Problem: input [8, 8192, 4096] f32 (8 simulated TP ranks), residual
[8192, 4096], norm_weight [4096].  reference = sum(input, axis=0) +
residual, then RMSNorm with gamma; returns (out, residual_out).

Sharding choice: instead of giving each core one rank shard and paying a
wire-level collective (~N bytes/rank over NeuronLink), we shard the TOKEN
axis: core j holds rows [j*1024, (j+1)*1024) of ALL 8 rank shards and sums
them locally.  Zero inter-core traffic; the kernel is purely HBM-bound.

Precision/traffic trade: all streaming tensors (rank shards, residual,
norm weight, both outputs) move through HBM as bf16; the RMSNorm
statistics (sum-of-squares, variance, rsqrt) are computed in f32.
Per-core traffic drops from 185 MB (pure f32) to 94 MB.  Error energy vs the f32
reference is ~2e-5 resid_var (~4e-3 norm relative error) — three orders
of magnitude inside the 2e-2 gate.  Host casts the bf16 results back to
f32 so returned dtypes match the reference.

Device-side layout: inputs are pre-tiled on the host into contiguous
[row-tile][rank][128][4096] blocks so every DMA is one fully-contiguous
1 MB burst.
"""

import sys
import types

import numpy as np
import ml_dtypes

import concourse.bass as bass
import concourse.tile as tile
from concourse import bacc, mybir
from concourse.bass_utils import run_bass_kernel_spmd


def _ensure_ntff_hook():
    """This container's antenv stub lacks axon_hooks; run_bass_kernel_spmd
    imports it whenever tracing is requested (e.g. BASS_TRACE=1).  Recreate
    it from trn_agent_boot so tracing works instead of crashing."""
    try:
        import antenv.axon_hooks  # noqa: F401

        return
    except ImportError:
        pass
    try:
        import antenv
        from trn_agent_boot.trn_boot import _ntff_profile_via_ctypes

        hook = _ntff_profile_via_ctypes("/opt/axon/libaxon_pjrt.so")
    except Exception:
        return
    mod = types.ModuleType("antenv.axon_hooks")
    state = {"hook": hook}
    mod.set_axon_ntff_profile_hook = lambda h: state.__setitem__("hook", h)
    mod.get_axon_ntff_profile_hook = lambda: state["hook"]
    antenv.axon_hooks = mod
    sys.modules["antenv.axon_hooks"] = mod


_ensure_ntff_hook()

TP = 8          # simulated tensor-parallel ranks (leading axis of input)
T = 8192        # tokens
H = 4096        # hidden
NCORES = 8
ROWS = T // NCORES     # 1024 token rows per core
RT = 128               # row tile = SBUF partition count
NT = ROWS // RT        # 8 row tiles per core
HALF = H // 2          # 2048
EPS = 1e-6

_FP32 = mybir.dt.float32
_BF16 = mybir.dt.bfloat16


def _build_nc() -> bass.Bass:
    # Bacc (not raw Bass): its compile() pass legalizes multi-semaphore
    # waits via event semaphores — walrus rejects >1 sync wait on most
    # compute ISA structs otherwise.
    nc = bacc.Bacc("TRN2", target_bir_lowering=False, debug=False, num_devices=NCORES)

    x = nc.declare_dram_parameter("x", [NT, TP, RT, H], _BF16, isOutput=False)
    res = nc.declare_dram_parameter("res", [NT, RT, H], _BF16, isOutput=False)
    w = nc.declare_dram_parameter("w", [RT, H], _BF16, isOutput=False)
    out = nc.declare_dram_parameter("out", [NT, RT, H], _BF16, isOutput=True)
    rout = nc.declare_dram_parameter("rout", [NT, RT, H], _BF16, isOutput=True)

    with tile.TileContext(nc) as tc:
        with (
            tc.tile_pool(name="wpool", bufs=1) as wpool,
            tc.tile_pool(name="inp", bufs=13) as inp,
            tc.tile_pool(name="resp", bufs=2) as resp,
            tc.tile_pool(name="accp", bufs=2) as accp,
            tc.tile_pool(name="sqp", bufs=1) as sqp,
            tc.tile_pool(name="outp", bufs=3) as outp,
            tc.tile_pool(name="statp", bufs=4) as statp,
        ):
            wtile = wpool.tile([RT, H], _BF16)
            nc.sync.dma_start(wtile[:], w[:, :])

            for t in range(NT):
                tiles = []
                for r in range(TP):
                    b = inp.tile([RT, H], _BF16, tag="inb")
                    nc.sync.dma_start(b[:], x[t, r])
                    tiles.append(b)
                rtile = resp.tile([RT, H], _BF16, tag="resb")
                nc.sync.dma_start(rtile[:], res[t])

                # binary-tree rank reduction in bf16 (DVE 2x mode); each
                # instruction depends on at most two producers, keeping the
                # codegen sync-wait count within ISA limits.
                acc = accp.tile([RT, H], _BF16, tag="acc")
                with nc.allow_low_precision(reason="bf16 streaming path"):
                    while len(tiles) > 1:
                        nxt = []
                        for k in range(0, len(tiles), 2):
                            nc.vector.tensor_add(
                                tiles[k][:], tiles[k][:], tiles[k + 1][:]
                            )
                            nxt.append(tiles[k])
                        tiles = nxt
                    # + residual -> acc (= residual_out, bf16)
                    nc.vector.tensor_add(acc[:], tiles[0][:], rtile[:])
                nc.sync.dma_start(rout[t], acc[:])

                # RMSNorm: var = mean(acc^2, -1); out = acc * rsqrt(var+eps) * w
                # statistics in f32
                sq = sqp.tile([RT, H], _BF16, tag="sq")
                ss = statp.tile([RT, 1], _FP32, tag="ss")
                # square's tensor output is dead — only accum_out (f32 row
                # sum of squares) is used.
                nc.scalar.activation(
                    sq[:],
                    acc[:],
                    mybir.ActivationFunctionType.Square,
                    accum_out=ss[:, 0:1],
                )
                ve = statp.tile([RT, 1], _FP32, tag="ve")
                nc.vector.tensor_scalar(
                    ve[:],
                    ss[:],
                    1.0 / H,
                    EPS,
                    op0=mybir.AluOpType.mult,
                    op1=mybir.AluOpType.add,
                )
                nc.vector.reciprocal(ve[:], ve[:])
                rs = statp.tile([RT, 1], _FP32, tag="rs")
                nc.scalar.sqrt(rs[:], ve[:])

                o = outp.tile([RT, H], _BF16, tag="ot")
                with nc.allow_low_precision(reason="bf16 streaming path"):
                    nc.vector.scalar_tensor_tensor(
                        o[:],
                        acc[:],
                        rs[:, 0:1],
                        wtile[:],
                        op0=mybir.AluOpType.mult,
                        op1=mybir.AluOpType.mult,
                    )
                nc.sync.dma_start(out[t], o[:])

    nc.compile()
    return nc


_NC_CACHE: dict[str, bass.Bass] = {}


def _get_nc() -> bass.Bass:
    if "nc" not in _NC_CACHE:
        _NC_CACHE["nc"] = _build_nc()
    return _NC_CACHE["nc"]


def _make_in_maps(input, residual, norm_weight):
    inp = np.asarray(input, dtype=np.float32)
    res = np.asarray(residual, dtype=np.float32)
    w = np.asarray(norm_weight, dtype=np.float32)
    wt = np.ascontiguousarray(np.broadcast_to(w[None, :], (RT, H))).astype(
        ml_dtypes.bfloat16
    )
    in_maps = []
    for j in range(NCORES):
        sl = slice(j * ROWS, (j + 1) * ROWS)
        # [TP, ROWS, H] -> [NT, TP, RT, H] contiguous bf16 blocks
        xj = inp[:, sl, :].reshape(TP, NT, RT, H).transpose(1, 0, 2, 3)
        xj = np.ascontiguousarray(xj).astype(ml_dtypes.bfloat16)
        rj = res[sl].reshape(NT, RT, H).astype(ml_dtypes.bfloat16)
        in_maps.append(
            {
                "x": xj,
                "res": rj,
                "w": wt,
            }
        )
    return in_maps


def run(input, residual, norm_weight, **spmd_kwargs):
    """Build + run; returns (out, residual_out, BassKernelResults)."""
    nc = _get_nc()
    in_maps = _make_in_maps(input, residual, norm_weight)
    r = run_bass_kernel_spmd(nc, in_maps, core_ids=list(range(NCORES)), **spmd_kwargs)
    out = np.concatenate(
        [r.results[j]["out"].reshape(ROWS, H).astype(np.float32) for j in range(NCORES)],
        axis=0,
    )
    rout = np.concatenate(
        [
            r.results[j]["rout"].reshape(ROWS, H).astype(np.float32)
            for j in range(NCORES)
        ],
        axis=0,
    )
    return out, rout, r


def kernel(input, residual, norm_weight):
    out, rout, _ = run(input, residual, norm_weight)
    return out, rout


# revision 20
# speedup vs baseline: 1.1603x; 1.0685x over previous
"""Fused AllReduce + residual-add + RMSNorm for TRN2 (8 NeuronCores).

Problem: input [8, 8192, 4096] f32 (8 simulated TP ranks), residual
[8192, 4096], norm_weight [4096].  reference = sum(input, axis=0) +
residual, then RMSNorm with gamma; returns (out, residual_out).

Sharding choice: instead of giving each core one rank shard and paying a
wire-level collective (~N bytes/rank over NeuronLink), we shard the TOKEN
axis: core j holds rows [j*1024, (j+1)*1024) of ALL 8 rank shards and sums
them locally.  Zero inter-core traffic; the kernel is purely HBM-bound.

Precision/traffic trade: all streaming tensors (rank shards, residual,
norm weight, both outputs) move through HBM as bf16; the RMSNorm
statistics (sum-of-squares, variance, rsqrt) are computed in f32.
Per-core traffic drops from 185 MB (pure f32) to 94 MB.  Error energy vs the f32
reference is ~2e-5 resid_var (~4e-3 norm relative error) — three orders
of magnitude inside the 2e-2 gate.  Host casts the bf16 results back to
f32 so returned dtypes match the reference.

Device-side layout: inputs are pre-tiled on the host into contiguous
[row-tile][rank][128][4096] blocks so every DMA is one fully-contiguous
1 MB burst.
"""

import sys
import types

import numpy as np
import ml_dtypes

import concourse.bass as bass
import concourse.tile as tile
from concourse import bacc, mybir
from concourse.bass_utils import run_bass_kernel_spmd


def _ensure_ntff_hook():
    """This container's antenv stub lacks axon_hooks; run_bass_kernel_spmd
    imports it whenever tracing is requested (e.g. BASS_TRACE=1).  Recreate
    it from trn_agent_boot so tracing works instead of crashing."""
    try:
        import antenv.axon_hooks  # noqa: F401

        return
    except ImportError:
        pass
    try:
        import antenv
        from trn_agent_boot.trn_boot import _ntff_profile_via_ctypes

        hook = _ntff_profile_via_ctypes("/opt/axon/libaxon_pjrt.so")
    except Exception:
        return
    mod = types.ModuleType("antenv.axon_hooks")
    state = {"hook": hook}
    mod.set_axon_ntff_profile_hook = lambda h: state.__setitem__("hook", h)
    mod.get_axon_ntff_profile_hook = lambda: state["hook"]
    antenv.axon_hooks = mod
    sys.modules["antenv.axon_hooks"] = mod


_ensure_ntff_hook()

TP = 8          # simulated tensor-parallel ranks (leading axis of input)
T = 8192        # tokens
H = 4096        # hidden
NCORES = 8
ROWS = T // NCORES     # 1024 token rows per core
RT = 128               # row tile = SBUF partition count
NT = ROWS // RT        # 8 row tiles per core
HALF = H // 2          # 2048
EPS = 1e-6

_FP32 = mybir.dt.float32
_BF16 = mybir.dt.bfloat16


def _build_nc() -> bass.Bass:
    # Bacc (not raw Bass): its compile() pass legalizes multi-semaphore
    # waits via event semaphores — walrus rejects >1 sync wait on most
    # compute ISA structs otherwise.
    nc = bacc.Bacc("TRN2", target_bir_lowering=False, debug=False, num_devices=NCORES)

    x = nc.declare_dram_parameter("x", [NT, TP, RT, H], _BF16, isOutput=False)
    res = nc.declare_dram_parameter("res", [NT, RT, H], _BF16, isOutput=False)
    w = nc.declare_dram_parameter("w", [RT, H], _BF16, isOutput=False)
    out = nc.declare_dram_parameter("out", [NT, RT, H], _BF16, isOutput=True)
    rout = nc.declare_dram_parameter("rout", [NT, RT, H], _BF16, isOutput=True)

    with tile.TileContext(nc) as tc:
        with (
            tc.tile_pool(name="wpool", bufs=1) as wpool,
            tc.tile_pool(name="inp", bufs=13) as inp,
            tc.tile_pool(name="resp", bufs=2) as resp,
            tc.tile_pool(name="accp", bufs=2) as accp,
            tc.tile_pool(name="sqp", bufs=1) as sqp,
            tc.tile_pool(name="outp", bufs=3) as outp,
            tc.tile_pool(name="statp", bufs=4) as statp,
        ):
            wtile = wpool.tile([RT, H], _BF16)
            nc.sync.dma_start(wtile[:], w[:, :])

            for t in range(NT):
                tiles = []
                for r in range(TP):
                    b = inp.tile([RT, H], _BF16, tag="inb")
                    # split rank loads across both HWDGE queue groups (SP +
                    # Activation): each group's completion handling taxes a
                    # different SDMA engine, so one engine isn't the lone
                    # serialization point.
                    eng = nc.sync if r % 2 == 0 else nc.scalar
                    eng.dma_start(b[:], x[t, r])
                    tiles.append(b)
                rtile = resp.tile([RT, H], _BF16, tag="resb")
                nc.scalar.dma_start(rtile[:], res[t])

                # binary-tree rank reduction in bf16 (DVE 2x mode); each
                # instruction depends on at most two producers, keeping the
                # codegen sync-wait count within ISA limits.
                acc = accp.tile([RT, H], _BF16, tag="acc")
                with nc.allow_low_precision(reason="bf16 streaming path"):
                    while len(tiles) > 1:
                        nxt = []
                        for k in range(0, len(tiles), 2):
                            nc.vector.tensor_add(
                                tiles[k][:], tiles[k][:], tiles[k + 1][:]
                            )
                            nxt.append(tiles[k])
                        tiles = nxt
                    # + residual -> acc (= residual_out, bf16)
                    nc.vector.tensor_add(acc[:], tiles[0][:], rtile[:])
                nc.sync.dma_start(rout[t], acc[:])

                # RMSNorm: var = mean(acc^2, -1); out = acc * rsqrt(var+eps) * w
                # statistics in f32
                sq = sqp.tile([RT, H], _BF16, tag="sq")
                ss = statp.tile([RT, 1], _FP32, tag="ss")
                # square's tensor output is dead — only accum_out (f32 row
                # sum of squares) is used.
                nc.scalar.activation(
                    sq[:],
                    acc[:],
                    mybir.ActivationFunctionType.Square,
                    accum_out=ss[:, 0:1],
                )
                ve = statp.tile([RT, 1], _FP32, tag="ve")
                nc.vector.tensor_scalar(
                    ve[:],
                    ss[:],
                    1.0 / H,
                    EPS,
                    op0=mybir.AluOpType.mult,
                    op1=mybir.AluOpType.add,
                )
                nc.vector.reciprocal(ve[:], ve[:])
                rs = statp.tile([RT, 1], _FP32, tag="rs")
                nc.scalar.sqrt(rs[:], ve[:])

                o = outp.tile([RT, H], _BF16, tag="ot")
                with nc.allow_low_precision(reason="bf16 streaming path"):
                    nc.vector.scalar_tensor_tensor(
                        o[:],
                        acc[:],
                        rs[:, 0:1],
                        wtile[:],
                        op0=mybir.AluOpType.mult,
                        op1=mybir.AluOpType.mult,
                    )
                nc.sync.dma_start(out[t], o[:])

    nc.compile()
    return nc


_NC_CACHE: dict[str, bass.Bass] = {}


def _get_nc() -> bass.Bass:
    if "nc" not in _NC_CACHE:
        _NC_CACHE["nc"] = _build_nc()
    return _NC_CACHE["nc"]


def _make_in_maps(input, residual, norm_weight):
    inp = np.asarray(input, dtype=np.float32)
    res = np.asarray(residual, dtype=np.float32)
    w = np.asarray(norm_weight, dtype=np.float32)
    wt = np.ascontiguousarray(np.broadcast_to(w[None, :], (RT, H))).astype(
        ml_dtypes.bfloat16
    )
    in_maps = []
    for j in range(NCORES):
        sl = slice(j * ROWS, (j + 1) * ROWS)
        # [TP, ROWS, H] -> [NT, TP, RT, H] contiguous bf16 blocks
        xj = inp[:, sl, :].reshape(TP, NT, RT, H).transpose(1, 0, 2, 3)
        xj = np.ascontiguousarray(xj).astype(ml_dtypes.bfloat16)
        rj = res[sl].reshape(NT, RT, H).astype(ml_dtypes.bfloat16)
        in_maps.append(
            {
                "x": xj,
                "res": rj,
                "w": wt,
            }
        )
    return in_maps


def run(input, residual, norm_weight, **spmd_kwargs):
    """Build + run; returns (out, residual_out, BassKernelResults)."""
    nc = _get_nc()
    in_maps = _make_in_maps(input, residual, norm_weight)
    r = run_bass_kernel_spmd(nc, in_maps, core_ids=list(range(NCORES)), **spmd_kwargs)
    out = np.concatenate(
        [r.results[j]["out"].reshape(ROWS, H).astype(np.float32) for j in range(NCORES)],
        axis=0,
    )
    rout = np.concatenate(
        [
            r.results[j]["rout"].reshape(ROWS, H).astype(np.float32)
            for j in range(NCORES)
        ],
        axis=0,
    )
    return out, rout, r


def kernel(input, residual, norm_weight):
    out, rout, _ = run(input, residual, norm_weight)
    return out, rout


# revision 21
# speedup vs baseline: 1.2359x; 1.0651x over previous
"""Fused AllReduce + residual-add + RMSNorm for TRN2 (8 NeuronCores).

Problem: input [8, 8192, 4096] f32 (8 simulated TP ranks), residual
[8192, 4096], norm_weight [4096].  reference = sum(input, axis=0) +
residual, then RMSNorm with gamma; returns (out, residual_out).

Sharding choice: instead of giving each core one rank shard and paying a
wire-level collective (~N bytes/rank over NeuronLink), we shard the TOKEN
axis: core j holds rows [j*1024, (j+1)*1024) of ALL 8 rank shards and sums
them locally.  Zero inter-core traffic; the kernel is purely HBM-bound.

Precision/traffic trade: all streaming tensors (rank shards, residual,
norm weight, both outputs) move through HBM as bf16; the RMSNorm
statistics (sum-of-squares, variance, rsqrt) are computed in f32.
Per-core traffic drops from 185 MB (pure f32) to 94 MB.  Error energy vs the f32
reference is ~2e-5 resid_var (~4e-3 norm relative error) — three orders
of magnitude inside the 2e-2 gate.  Host casts the bf16 results back to
f32 so returned dtypes match the reference.

Device-side layout: inputs are pre-tiled on the host into contiguous
[row-tile][rank][128][4096] blocks so every DMA is one fully-contiguous
1 MB burst.
"""

import sys
import types

import numpy as np
import ml_dtypes

import concourse.bass as bass
import concourse.tile as tile
from concourse import bacc, mybir
from concourse.bass_utils import run_bass_kernel_spmd


def _ensure_ntff_hook():
    """This container's antenv stub lacks axon_hooks; run_bass_kernel_spmd
    imports it whenever tracing is requested (e.g. BASS_TRACE=1).  Recreate
    it from trn_agent_boot so tracing works instead of crashing."""
    try:
        import antenv.axon_hooks  # noqa: F401

        return
    except ImportError:
        pass
    try:
        import antenv
        from trn_agent_boot.trn_boot import _ntff_profile_via_ctypes

        hook = _ntff_profile_via_ctypes("/opt/axon/libaxon_pjrt.so")
    except Exception:
        return
    mod = types.ModuleType("antenv.axon_hooks")
    state = {"hook": hook}
    mod.set_axon_ntff_profile_hook = lambda h: state.__setitem__("hook", h)
    mod.get_axon_ntff_profile_hook = lambda: state["hook"]
    antenv.axon_hooks = mod
    sys.modules["antenv.axon_hooks"] = mod


_ensure_ntff_hook()

TP = 8          # simulated tensor-parallel ranks (leading axis of input)
T = 8192        # tokens
H = 4096        # hidden
NCORES = 8
ROWS = T // NCORES     # 1024 token rows per core
RT = 128               # row tile = SBUF partition count
NT = ROWS // RT        # 8 row tiles per core
HALF = H // 2          # 2048
EPS = 1e-6

_FP32 = mybir.dt.float32
_BF16 = mybir.dt.bfloat16


def _build_nc() -> bass.Bass:
    # Bacc (not raw Bass): its compile() pass legalizes multi-semaphore
    # waits via event semaphores — walrus rejects >1 sync wait on most
    # compute ISA structs otherwise.
    nc = bacc.Bacc("TRN2", target_bir_lowering=False, debug=False, num_devices=NCORES)

    x = nc.declare_dram_parameter("x", [NT, TP, RT, H], _BF16, isOutput=False)
    res = nc.declare_dram_parameter("res", [NT, RT, H], _BF16, isOutput=False)
    w = nc.declare_dram_parameter("w", [RT, H], _BF16, isOutput=False)
    out = nc.declare_dram_parameter("out", [NT, RT, H], _BF16, isOutput=True)
    rout = nc.declare_dram_parameter("rout", [NT, RT, H], _BF16, isOutput=True)

    with tile.TileContext(nc) as tc:
        with (
            tc.tile_pool(name="wpool", bufs=1) as wpool,
            tc.tile_pool(name="inp", bufs=13) as inp,
            tc.tile_pool(name="resp", bufs=2) as resp,
            tc.tile_pool(name="accp", bufs=2) as accp,
            tc.tile_pool(name="sqp", bufs=1) as sqp,
            tc.tile_pool(name="outp", bufs=3) as outp,
            tc.tile_pool(name="statp", bufs=4) as statp,
        ):
            wtile = wpool.tile([RT, H], _BF16)
            nc.sync.dma_start(wtile[:], w[:, :])

            for t in range(NT):
                tiles = []
                for r in range(TP):
                    b = inp.tile([RT, H], _BF16, tag="inb")
                    # split rank loads across both HWDGE queue groups (SP +
                    # Activation): each group's completion handling taxes a
                    # different SDMA engine, so one engine isn't the lone
                    # serialization point.
                    eng = nc.sync if r % 2 == 0 else nc.scalar
                    eng.dma_start(b[:], x[t, r])
                    tiles.append(b)
                rtile = resp.tile([RT, H], _BF16, tag="resb")
                nc.scalar.dma_start(rtile[:], res[t])

                # binary-tree rank reduction in bf16 (DVE 2x mode); each
                # instruction depends on at most two producers, keeping the
                # codegen sync-wait count within ISA limits.  The last row
                # tile runs its reduction tail + epilogue in column chunks so
                # the final stores overlap the final adds (shorter kernel
                # tail); earlier tiles use full-width ops (fewer
                # instructions, contiguous stores).
                last = t == NT - 1
                nch = 4 if last else 1
                CW = H // nch
                acc = accp.tile([RT, H], _BF16, tag="acc")
                with nc.allow_low_precision(reason="bf16 streaming path"):
                    while len(tiles) > 2:
                        nxt = []
                        for k in range(0, len(tiles), 2):
                            nc.vector.tensor_add(
                                tiles[k][:], tiles[k][:], tiles[k + 1][:]
                            )
                            nxt.append(tiles[k])
                        tiles = nxt
                    for c in range(nch):
                        cc = slice(c * CW, (c + 1) * CW)
                        nc.vector.tensor_add(
                            tiles[0][:, cc], tiles[0][:, cc], tiles[1][:, cc]
                        )
                        # + residual -> acc (= residual_out, bf16)
                        nc.vector.tensor_add(
                            acc[:, cc], tiles[0][:, cc], rtile[:, cc]
                        )
                        nc.sync.dma_start(
                            rout[t].rearrange("p (k c) -> p k c", k=nch)[:, c, :],
                            acc[:, cc],
                        )

                # RMSNorm: var = mean(acc^2, -1); out = acc * rsqrt(var+eps) * w
                # statistics in f32
                sq = sqp.tile([RT, H], _BF16, tag="sq")
                ss = statp.tile([RT, 4], _FP32, tag="ss")
                # square's tensor output is dead — only accum_out (f32 row
                # sum of squares) is used.
                for c in range(nch):
                    cc = slice(c * CW, (c + 1) * CW)
                    nc.scalar.activation(
                        sq[:, cc],
                        acc[:, cc],
                        mybir.ActivationFunctionType.Square,
                        accum_out=ss[:, c : c + 1],
                    )
                ve = statp.tile([RT, 1], _FP32, tag="ve")
                if nch > 1:
                    s1 = statp.tile([RT, 1], _FP32, tag="s1")
                    nc.vector.tensor_reduce(
                        s1[:],
                        ss[:, 0:nch],
                        axis=mybir.AxisListType.X,
                        op=mybir.AluOpType.add,
                    )
                else:
                    s1 = ss
                nc.vector.tensor_scalar(
                    ve[:],
                    s1[:, 0:1],
                    1.0 / H,
                    EPS,
                    op0=mybir.AluOpType.mult,
                    op1=mybir.AluOpType.add,
                )
                nc.vector.reciprocal(ve[:], ve[:])
                rs = statp.tile([RT, 1], _FP32, tag="rs")
                nc.scalar.sqrt(rs[:], ve[:])

                o = outp.tile([RT, H], _BF16, tag="ot")
                with nc.allow_low_precision(reason="bf16 streaming path"):
                    for c in range(nch):
                        cc = slice(c * CW, (c + 1) * CW)
                        nc.vector.scalar_tensor_tensor(
                            o[:, cc],
                            acc[:, cc],
                            rs[:, 0:1],
                            wtile[:, cc],
                            op0=mybir.AluOpType.mult,
                            op1=mybir.AluOpType.mult,
                        )
                        nc.scalar.dma_start(
                            out[t].rearrange("p (k c) -> p k c", k=nch)[:, c, :],
                            o[:, cc],
                        )

    nc.compile()
    return nc


_NC_CACHE: dict[str, bass.Bass] = {}


def _get_nc() -> bass.Bass:
    if "nc" not in _NC_CACHE:
        _NC_CACHE["nc"] = _build_nc()
    return _NC_CACHE["nc"]


def _make_in_maps(input, residual, norm_weight):
    inp = np.asarray(input, dtype=np.float32)
    res = np.asarray(residual, dtype=np.float32)
    w = np.asarray(norm_weight, dtype=np.float32)
    wt = np.ascontiguousarray(np.broadcast_to(w[None, :], (RT, H))).astype(
        ml_dtypes.bfloat16
    )
    in_maps = []
    for j in range(NCORES):
        sl = slice(j * ROWS, (j + 1) * ROWS)
        # [TP, ROWS, H] -> [NT, TP, RT, H] contiguous bf16 blocks
        xj = inp[:, sl, :].reshape(TP, NT, RT, H).transpose(1, 0, 2, 3)
        xj = np.ascontiguousarray(xj).astype(ml_dtypes.bfloat16)
        rj = res[sl].reshape(NT, RT, H).astype(ml_dtypes.bfloat16)
        in_maps.append(
            {
                "x": xj,
                "res": rj,
                "w": wt,
            }
        )
    return in_maps


def run(input, residual, norm_weight, **spmd_kwargs):
    """Build + run; returns (out, residual_out, BassKernelResults)."""
    nc = _get_nc()
    in_maps = _make_in_maps(input, residual, norm_weight)
    r = run_bass_kernel_spmd(nc, in_maps, core_ids=list(range(NCORES)), **spmd_kwargs)
    out = np.concatenate(
        [r.results[j]["out"].reshape(ROWS, H).astype(np.float32) for j in range(NCORES)],
        axis=0,
    )
    rout = np.concatenate(
        [
            r.results[j]["rout"].reshape(ROWS, H).astype(np.float32)
            for j in range(NCORES)
        ],
        axis=0,
    )
    return out, rout, r


def kernel(input, residual, norm_weight):
    out, rout, _ = run(input, residual, norm_weight)
    return out, rout
